# revision 10
# baseline (speedup 1.0000x reference)
"""Trainium2 Bass kernel for nn_MultiHeadAttention_67284957659561.

Full-input contract: kernel(**inputs) takes the unsharded tensors from
setup_inputs() and returns the full (2, 2048, 1024) float32 output.

Sharding: 8 cores = 2 batches x 4 head-groups (4 heads per core, processed
as 2 pairs of 2 heads).  Per core, everything runs on-device except:
  - input layout prep (transposes / slicing, pure numpy)
  - the final sum of the 4 partial outputs per batch (+ bv@Wo.T + bo)

Device algorithm per core (batch b, heads h0..h0+3):
  QT/KT = W @ x.T computed in transposed layout [head_dim, seq] so the
  scores matmuls contract over d_k on partitions.  Heads are stored in
  pairs [128 = 2*64 dims, seq], letting two K=64 scores matmuls run
  concurrently via tile_position row packing.  V is computed in natural
  layout [seq, d_k] with a ones column appended, so attn@V (lhsT = V_aug)
  yields H.T with the softmax denominators as row 64 for free (no max
  subtraction needed: |scores| < ~50 so exp stays in fp32 range).
  Normalization multiplies columns by 1/denominator via a gpsimd
  partition_broadcast + one DVE multiply, writing H.T which is exactly
  the lhsT the output projection needs.  All matmuls use float32r
  (11-bit mantissa RNE inputs, fp32 accumulation).
"""

import sys

sys.path.insert(0, "/opt/trn_rl_repo")

import numpy as np

import concourse.bass as bass  # noqa: F401  (registers types)
import concourse.mybir as mybir
import concourse.tile as tile
from concourse import bacc
from concourse.bass_utils import run_bass_kernel_spmd

dt = mybir.dt

B, S, D, H, DK = 2, 2048, 1024, 16, 64
N_CORES = 8
HEADS_PER_CORE = H // (N_CORES // B)  # 4
PAIRS = HEADS_PER_CORE // 2  # 2
ROWS = HEADS_PER_CORE * DK  # 256 (this core's slice of d_model)
KT_TILES = S // 128  # 16 k tiles
QW = 512  # q window processed per scores round
NQW = S // QW  # 4
NEG_INF = -1e9

_cached = {}


def build_nc(repeat=1):
    """Build the SPMD Bass program (same for every core)."""
    nc = bacc.Bacc("TRN2", target_bir_lowering=False, debug=False,
                   num_devices=N_CORES)
    f32r, f32 = dt.float32r, dt.float32

    xT_d = nc.dram_tensor("xT", [D, S], f32r, kind="ExternalInput").ap()
    wq_d = nc.dram_tensor("wq", [D, ROWS], f32r, kind="ExternalInput").ap()
    wk_d = nc.dram_tensor("wk", [D, ROWS], f32r, kind="ExternalInput").ap()
    wv_d = nc.dram_tensor("wv", [D, ROWS], f32r, kind="ExternalInput").ap()
    wo_d = nc.dram_tensor("wo", [ROWS, D], f32r, kind="ExternalInput").ap()
    qkb_d = nc.dram_tensor("qkb", [128, 2 * PAIRS], f32, kind="ExternalInput").ap()
    kb_d = nc.dram_tensor("kb", [128, KT_TILES], f32, kind="ExternalInput").ap()
    out_d = nc.dram_tensor("out", [S, D], f32, kind="ExternalOutput").ap()

    with tile.TileContext(nc) as tc:
        import contextlib
        with contextlib.ExitStack() as ctx:
            const = ctx.enter_context(tc.tile_pool(name="const", bufs=1))
            expp = ctx.enter_context(tc.tile_pool(name="expp", bufs=4))
            smal = ctx.enter_context(tc.tile_pool(name="smal", bufs=4))
            outp = ctx.enter_context(tc.tile_pool(name="outp", bufs=3))
            psS = ctx.enter_context(tc.tile_pool(name="psS", bufs=2, space="PSUM"))
            psH = ctx.enter_context(tc.tile_pool(name="psH", bufs=4, space="PSUM"))

            # ---- resident tensors -------------------------------------
            xT = const.tile([128, D // 128, S], f32r)          # 64KB/part
            wq = const.tile([128, D // 128, ROWS], f32r)       # 8KB/part
            wk = const.tile([128, D // 128, ROWS], f32r)
            wv = const.tile([128, D // 128, ROWS], f32r)
            wo = const.tile([128, ROWS // 128, D], f32r)       # 8KB/part
            qkb = const.tile([128, 2 * PAIRS], f32)
            kb = const.tile([128, KT_TILES], f32)
            QT = const.tile([128, PAIRS, S], f32r)             # 16KB/part
            KT = const.tile([128, PAIRS, S], f32r)
            # V natural layout + ones col: [seq%128, pair, ktile, head, 65]
            V = const.tile([128, PAIRS, KT_TILES, 2, DK + 1], f32r)
            HT = const.tile([128, PAIRS, S], f32r)             # 16KB/part

            # DMA order = need order: biases, then per-chunk (wk, wq, xT)
            # for the head projections, wv next (V starts ~27us in), wo last
            # (output projection starts much later).
            nc.sync.dma_start(qkb[:], qkb_d[:, :])
            nc.sync.dma_start(kb[:], kb_d[:, :])
            for c in range(D // 128):
                nc.sync.dma_start(wk[:, c, :], wk_d[c * 128:(c + 1) * 128, :])
                nc.sync.dma_start(wq[:, c, :], wq_d[c * 128:(c + 1) * 128, :])
                nc.sync.dma_start(xT[:, c, :], xT_d[c * 128:(c + 1) * 128, :])
            for c in range(D // 128):
                nc.sync.dma_start(wv[:, c, :], wv_d[c * 128:(c + 1) * 128, :])
            for c in range(ROWS // 128):
                nc.sync.dma_start(wo[:, c, :], wo_d[c * 128:(c + 1) * 128, :])
            # ones columns of V_aug (exactly representable in f32r)
            nc.gpsimd.memset(V[:, :, :, :, DK:DK + 1].bitcast(dt.float32), 1.0)

            def proj_group_items(w, dest, pair, qc, half, proj):
                """A [128,512] projection psum split into 2-matmul drip items."""
                m0 = pair * 128
                q0 = qc * 1024 + half * 512
                box = {}

                def mk(k0):
                    def item():
                        if k0 == 0:
                            box["ps"] = psH.tile([128, 512], f32, tag="acc", name="pjps")
                        for k in (k0, k0 + 1):
                            nc.tensor.matmul(
                                box["ps"][:], w[:, k, m0:m0 + 128],
                                xT[:, k, q0:q0 + 512],
                                start=(k == 0), stop=(k == D // 128 - 1))
                        if k0 + 2 == D // 128:
                            nc.vector.tensor_scalar_add(
                                dest[:, pair, q0:q0 + 512], box["ps"][:],
                                qkb[:, 2 * pair + proj:2 * pair + proj + 1])
                    return item
                return [mk(k0) for k0 in range(0, D // 128, 2)]

            def emit_proj_group(w, dest, pair, qc, half, proj):
                for it in proj_group_items(w, dest, pair, qc, half, proj):
                    it()

            def emit_v(sc):
                """V projection (both pairs) for one seq chunk, natural layout."""
                ps = psH.tile([128, ROWS], f32, tag="acc")
                for k in range(D // 128):
                    nc.tensor.matmul(
                        ps[:], xT[:, k, sc * 128:(sc + 1) * 128], wv[:, k, :],
                        start=(k == 0), stop=(k == D // 128 - 1))
                for pair in range(PAIRS):
                    src = ps[:, pair * 128:(pair + 1) * 128]
                    nc.vector.tensor_copy(
                        V[:, pair, sc, :, 0:DK],
                        src.rearrange("p (h d) -> p h d", h=2))

            def phase4_items(qw):
                """Output projection for window qw as drip items (2 per psum)."""
                items = []
                for sub in range(QW // 128):
                    qt0 = qw * QW + sub * 128
                    for nh in range(D // 512):
                        box = {}

                        def mm(qt0=qt0, nh=nh, box=box):
                            box["pp"] = psH.tile([128, 512], f32, tag="acc", name="pp")
                            for pair in range(PAIRS):
                                nc.tensor.matmul(
                                    box["pp"][:], HT[:, pair, qt0:qt0 + 128],
                                    wo[:, pair, nh * 512:(nh + 1) * 512],
                                    start=(pair == 0), stop=(pair == PAIRS - 1))

                        def st(qt0=qt0, nh=nh, box=box):
                            po = outp.tile([128, 512], f32, tag="po")
                            nc.vector.tensor_copy(po[:], box["pp"][:])
                            nc.sync.dma_start(
                                out_d[qt0:qt0 + 128, nh * 512:(nh + 1) * 512],
                                po[:])
                        items += [mm, st]
                return items

            def rep_body(_i=None, unroll=None):
                # Warm the ACT exp table immediately.
                dummy = smal.tile([1, 2], f32, tag="rec")
                nc.gpsimd.memset(dummy[:], 0.0)
                nc.scalar.activation(dummy[:], dummy[:],
                                     mybir.ActivationFunctionType.Exp)

                # Head (runs under the xT input-DMA shadow): pair-0 K, first
                # half of pair-0 Q, V(0..1).
                for qc in range(2):
                    for half in range(2):
                        emit_proj_group(wk, KT, 0, qc, half, 1)
                for half in range(2):
                    emit_proj_group(wq, QT, 0, 0, half, 0)
                emit_v(0)
                emit_v(1)

                # Drip queue: small PE work items injected one per kt
                # iteration so the ACT exp stream never starves.
                drip = []
                for half in range(2):  # Q pair0 qc=1 (needed by window 2)
                    drip += proj_group_items(wq, QT, 0, 1, half, 0)
                for proj, w, dest in ((0, wq, QT), (1, wk, KT)):  # pair 1
                    for qc in range(2):
                        for half in range(2):
                            drip += proj_group_items(w, dest, 1, qc, half, proj)

                def window(pair, qw, vfeed=False):
                    q0 = qw * QW
                    accA = psH.tile([DK + 1, QW], f32, tag="acc")
                    accB = psH.tile([DK + 1, QW], f32, tag="acc")
                    for kt in range(KT_TILES):
                        k0 = kt * 128
                        ps = psS.tile([128, 1024], f32, tag="ps")
                        nc.tensor.matmul(
                            ps[:, 0:QW],
                            KT[0:64, pair, k0:k0 + 128],
                            QT[0:64, pair, q0:q0 + QW],
                            start=True, stop=True, tile_position=(0, 0))
                        nc.tensor.matmul(
                            ps[:, QW:2 * QW],
                            KT[64:128, pair, k0:k0 + 128],
                            QT[64:128, pair, q0:q0 + QW],
                            start=True, stop=True, tile_position=(64, 0))
                        et = expp.tile([128, 1024], f32r, tag="et")
                        nc.scalar.activation(
                            et[:], ps[:], mybir.ActivationFunctionType.Exp,
                            bias=kb[:, kt:kt + 1])
                        nc.tensor.matmul(
                            accA[:], V[:, pair, kt, 0, :], et[:, 0:QW],
                            start=(kt == 0), stop=(kt == KT_TILES - 1))
                        nc.tensor.matmul(
                            accB[:], V[:, pair, kt, 1, :], et[:, QW:2 * QW],
                            start=(kt == 0), stop=(kt == KT_TILES - 1))
                        if vfeed and kt + 2 < KT_TILES:
                            emit_v(kt + 2)
                        elif drip:
                            drip.pop(0)()
                    for h, acc in ((0, accA), (1, accB)):
                        rec = smal.tile([1, QW], f32, tag="rec")
                        nc.vector.reciprocal(rec[:], acc[DK:DK + 1, :])
                        bc = smal.tile([DK, QW], f32, tag="bc")
                        nc.gpsimd.partition_broadcast(bc[:], rec[:])
                        nc.vector.tensor_mul(
                            HT[h * DK:(h + 1) * DK, pair, q0:q0 + QW],
                            acc[0:DK, :], bc[:])

                window(0, 0, vfeed=True)
                for qw in range(1, NQW):
                    window(0, qw)
                for qw in range(NQW):
                    window(1, qw)
                    drip += phase4_items(qw)
                while drip:
                    drip.pop(0)()

            if repeat == 1:
                rep_body()
            else:
                with tc.For_i(0, repeat, 1) as i:
                    rep_body(i)

    nc.compile()
    return nc


def _prep_core_inputs(c, x, mask, Wq, bq, Wk, bk, Wv, bv, Wo, bo):
    b = c // (N_CORES // B)
    g = c % (N_CORES // B)
    rows = slice(g * ROWS, (g + 1) * ROWS)
    xT = np.ascontiguousarray(x[b].T).astype(np.float32)
    wq = np.ascontiguousarray(Wq[rows, :].T).astype(np.float32)
    wk = np.ascontiguousarray(Wk[rows, :].T).astype(np.float32)
    wv = np.ascontiguousarray(Wv[rows, :].T).astype(np.float32)
    wo = np.ascontiguousarray(Wo[:, rows].T).astype(np.float32)
    qkb = np.stack(
        [np.asarray(v, np.float32)
         for p in range(PAIRS)
         for v in (bq[rows][p * 128:(p + 1) * 128],
                   bk[rows][p * 128:(p + 1) * 128])],
        axis=1).astype(np.float32)
    kbias = np.where(np.asarray(mask[b, 0, 0]) == 0, np.float32(NEG_INF),
                     np.float32(0.0)).astype(np.float32)
    kb = np.ascontiguousarray(kbias.reshape(KT_TILES, 128).T)
    return {"xT": xT, "wq": wq, "wk": wk, "wv": wv, "wo": wo,
            "qkb": qkb, "kb": kb}


def kernel(x, mask, Wq, bq, Wk, bk, Wv, bv, Wo, bo):
    x = np.asarray(x)
    if "nc" not in _cached:
        _cached["nc"] = build_nc()
    nc = _cached["nc"]
    args = (np.asarray(mask), np.asarray(Wq), np.asarray(bq), np.asarray(Wk),
            np.asarray(bk), np.asarray(Wv), np.asarray(bv), np.asarray(Wo),
            np.asarray(bo))
    in_maps = [_prep_core_inputs(c, x, *args) for c in range(N_CORES)]
    res = run_bass_kernel_spmd(nc, in_maps, list(range(N_CORES))).results

    Wo_np, bv_np, bo_np = np.asarray(Wo), np.asarray(bv), np.asarray(bo)
    const_vec = (bv_np @ Wo_np.T + bo_np).astype(np.float32)
    out = np.zeros((B, S, D), np.float32)
    per_b = N_CORES // B
    for bidx in range(B):
        acc = np.zeros((S, D), np.float64)
        for g in range(per_b):
            acc += res[bidx * per_b + g]["out"]
        out[bidx] = (acc + const_vec).astype(np.float32)
    return out
